# revision 14
# baseline (speedup 1.0000x reference)
"""Trainium2 Bass kernel for the CMIN video encoder (2x banded MHA + BiGRU).

v2: the axon execution path charges ~30-150us of wall time PER STATIC
INSTRUCTION (measured; hardware For_i loops execute dynamic iterations at
full speed with no such charge). So this build packs the whole model into
a few hundred static instructions using For_i hardware loops:

- batch loop (8 per core) x per-head loop for both attention layers, with
  weights copied from a resident packed bank into fixed scratch so matmul
  lhsT offsets stay static (ldweights cannot take register offsets).
- banded softmax folded into the score PSUM via an identity-matmul mask add
  (band = 0 / -1e9), then one Exp activation; denominators via ones-matmul.
- BiGRU: forward chain iterates t ascending, backward chain iterates t
  descending over the SAME step program -- pack_padded semantics fall out of
  a +30 bias on the z gate for padded steps (z=1 => carry), so there are no
  reversals and no gathers. h lives in fp16 "ysb" ring tiles [258 blocks of
  (hc,b)] whose block 0/257 stay zero as the initial state for both ends.
- everything stays in SBUF between phases; ~15 DMAs total, all of them
  whole-[128,N] images prepacked on the host.
"""

import os

import ml_dtypes
import numpy as np

import concourse.bacc as bacc
import concourse.mybir as mybir
import concourse.tile as tile
from concourse.bass import ds
from concourse.bass_utils import run_bass_kernel_spmd

B, T, D = 64, 256, 1024
H, DK = 8, D // 8
HID = 512
GH = HID >> 1            # 256
ATTN_WIDTH = 3
NL = 2
NCORES = 8
BC = B // NCORES         # 8 batches per core
KC = D // 128            # 8
OC6 = 6                  # 768 / 128 gate chunks
HC = 2                   # 256 / 128 hidden chunks
TB = T + 2               # ysb time blocks incl zero blocks 0 and 257
SCALE = 1.0 / float(np.sqrt(DK))

F32 = mybir.dt.float32
BF16 = mybir.dt.bfloat16
FP16 = mybir.dt.float16
AF = mybir.ActivationFunctionType


def _build(repeat: int = 1, phases: str = "all"):
    nc = bacc.Bacc("TRN2", num_devices=NCORES)

    x_d = nc.dram_tensor("x_img", [128, KC * BC * T], BF16, kind="ExternalInput")
    wl_d = [
        nc.dram_tensor(f"wl{l}", [128, 4 * 8 * KC * 128], BF16, kind="ExternalInput")
        for l in range(NL)
    ]
    wg_d = nc.dram_tensor("wg", [128, 2 * OC6 * KC * 128], BF16, kind="ExternalInput")
    whh_d = nc.dram_tensor("whh", [128, 2 * OC6 * HC * 128], FP16,
                           kind="ExternalInput")
    iden_d = nc.dram_tensor("iden", [128, 128], BF16, kind="ExternalInput")
    ones_d = nc.dram_tensor("ones", [128, 128], BF16, kind="ExternalInput")
    band_d = nc.dram_tensor("band", [128, 2 * T], BF16, kind="ExternalInput")
    zmask_d = nc.dram_tensor("zmask", [128, 2 * OC6 * BC * T], BF16,
                             kind="ExternalInput")
    ymask_d = nc.dram_tensor("ymask", [128, TB * 16], FP16, kind="ExternalInput")
    yout = nc.dram_tensor("yout", [2, 128, TB * 16], FP16, kind="ExternalOutput")

    with (
        nc.allow_low_precision(reason="bf16/fp16 compute within tolerance"),
        tile.TileContext(nc) as tc,
        tc.tile_pool(name="persist", bufs=1) as ppool,
    ):
        def scope(name):
            sid, _ = nc.enter_named_scope(name, False)
            return (name, sid)

        def unscope(s):
            nc.leave_named_scope(s[0], s[1], False)

        # ---- persistent tiles -------------------------------------------
        x_t = ppool.tile([128, KC * BC * T], BF16, name="x_t")
        iden_t = ppool.tile([128, 128], BF16, name="iden_t")
        ones_t = ppool.tile([128, 128], BF16, name="ones_t")
        band_t = ppool.tile([128, 2 * T], BF16, name="band_t")
        ysb_f = ppool.tile([128, TB * 16], FP16, name="ysb_f")
        ysb_b = ppool.tile([128, TB * 16], FP16, name="ysb_b")
        ymask_t = ppool.tile([128, TB * 16], FP16, name="ymask_t")

        nc.sync.dma_start(x_t[:], x_d[:])
        nc.sync.dma_start(iden_t[:], iden_d[:])
        nc.sync.dma_start(ones_t[:], ones_d[:])
        nc.sync.dma_start(band_t[:], band_d[:])
        nc.sync.dma_start(ymask_t[:], ymask_d[:])

        def attn_phase():
            with (
                tc.tile_pool(name="attn", bufs=1) as apool,
                tc.tile_pool(name="psA", bufs=1, space="PSUM") as psAp,
            ):
                wl_t = apool.tile([128, 4 * 8 * KC * 128], BF16, name="wl_t")
                qkv_b = apool.tile([128, 3 * H * T], BF16, name="qkv_b")
                ao_b = apool.tile([128, H * T], BF16, name="ao_b")
                kscr = apool.tile([128, T], BF16, name="kscr")
                vscr = apool.tile([128, T], F32, name="vscr")
                vtscr = apool.tile([128, T], BF16, name="vtscr")
                wcopy = apool.tile([128, 1024], BF16, name="wcopy")
                pm = apool.tile([128, 2 * T], BF16, name="pm")
                rr = apool.tile([128, T], F32, name="rr")
                psA = psAp.tile([128, T], F32, name="psA", tag="psA")
                ps_sc = psAp.tile([128, 2 * T], F32, name="ps_sc", tag="ps_sc")
                ps_dn = psAp.tile([128, T], F32, name="ps_dn", tag="ps_dn")
                ps_av = psAp.tile([128, T], F32, name="ps_av", tag="ps_av")
                ps_vt = psAp.tile([128, 128], F32, name="ps_vt", tag="ps_vt")
                iden32 = apool.tile([128, 128], F32, name="iden32")
                nc.vector.tensor_copy(iden32[:], iden_t[:])

                for l in range(NL):
                    s = scope(f"L{l}")
                    nc.sync.dma_start(wl_t[:], wl_d[l][:])
                    with tc.For_i(0, BC) as bi:
                        # ---- q, k, v projections for batch bi ----
                        with tc.For_i(0, 24) as wo:
                            nc.vector.tensor_copy(
                                wcopy[:], wl_t[:, ds(wo * 1024, 1024)]
                            )
                            for kc in range(KC):
                                nc.tensor.matmul(
                                    psA[:],
                                    wcopy[:, kc * 128:(kc + 1) * 128],
                                    x_t[:, ds(kc * BC * T + bi * T, T)],
                                    start=(kc == 0),
                                    stop=(kc == KC - 1),
                                )
                            nc.scalar.activation(
                                qkv_b[:, ds(wo * T, T)], psA[:], AF.Copy
                            )
                        # ---- banded attention per head ----
                        with tc.For_i(0, H) as h:
                            nc.vector.tensor_copy(
                                kscr[:], qkv_b[:, ds(H * T + h * T, T)]
                            )
                            nc.vector.tensor_copy(
                                vscr[:], qkv_b[:, ds(2 * H * T + h * T, T)]
                            )
                            # vscr is [dk, keys]; av needs keys on partitions
                            for c in range(2):
                                nc.tensor.transpose(
                                    ps_vt[:], vscr[:, c * 128:(c + 1) * 128],
                                    iden32[:],
                                )
                                nc.vector.tensor_copy(
                                    vtscr[:, c * 128:(c + 1) * 128], ps_vt[:]
                                )
                            for c in range(2):
                                nc.tensor.matmul(
                                    ps_sc[:, c * T:(c + 1) * T],
                                    kscr[:, c * 128:(c + 1) * 128],
                                    qkv_b[:, ds(h * T, T)],
                                    start=True, stop=False,
                                )
                                nc.tensor.matmul(
                                    ps_sc[:, c * T:(c + 1) * T],
                                    iden_t[:],
                                    band_t[:, c * T:(c + 1) * T],
                                    start=False, stop=True,
                                )
                            nc.scalar.activation(pm[:], ps_sc[:], AF.Exp,
                                                 scale=SCALE)
                            for c in range(2):
                                nc.tensor.matmul(
                                    ps_dn[:], ones_t[:], pm[:, c * T:(c + 1) * T],
                                    start=(c == 0), stop=(c == 1),
                                )
                            nc.vector.reciprocal(rr[:], ps_dn[:])
                            for c in range(2):
                                nc.tensor.matmul(
                                    ps_av[:], vtscr[:, c * 128:(c + 1) * 128],
                                    pm[:, c * T:(c + 1) * T],
                                    start=(c == 0), stop=(c == 1),
                                )
                            nc.vector.tensor_mul(
                                ao_b[:, ds(h * T, T)], ps_av[:], rr[:]
                            )
                        # ---- output projection + residual ----
                        with tc.For_i(0, 8) as oc2:
                            nc.vector.tensor_copy(
                                wcopy[:],
                                wl_t[:, ds(3 * 8192 + oc2 * 1024, 1024)],
                            )
                            for kc in range(KC):
                                nc.tensor.matmul(
                                    psA[:],
                                    wcopy[:, kc * 128:(kc + 1) * 128],
                                    ao_b[:, kc * T:(kc + 1) * T],
                                    start=(kc == 0),
                                    stop=(kc == KC - 1),
                                )
                            xsl = x_t[:, ds(oc2 * BC * T + bi * T, T)]
                            nc.vector.tensor_add(xsl, psA[:], xsl)
                    unscope(s)

        def gru_phase():
            with (
                tc.tile_pool(name="gru", bufs=1) as gpool,
                tc.tile_pool(name="psG", bufs=1, space="PSUM") as psGp,
            ):
                wg_t = gpool.tile([128, 2 * OC6 * KC * 128], BF16, name="wg_t")
                whh_t = gpool.tile([128, 2 * OC6 * HC * 128], FP16, name="whh_t")
                zmask_t = gpool.tile([128, 2 * OC6 * BC * T], BF16,
                                     name="zmask_t")
                gx_t = gpool.tile([128, 2 * OC6 * BC * T], BF16, name="gx_t")
                wcopy2 = gpool.tile([128, 1024], BF16, name="wcopy2")
                psA2 = psGp.tile([128, T], F32, name="psA2", tag="psA2")
                ps_g = psGp.tile([128, 96], F32, name="ps_g", tag="ps_g")

                grz = [gpool.tile([128, 32], F32, name=f"grz{d}") for d in range(2)]
                rz = [gpool.tile([128, 32], F32, name=f"rz{d}") for d in range(2)]
                t1 = [gpool.tile([128, 16], F32, name=f"t1{d}") for d in range(2)]
                t2 = [gpool.tile([128, 16], F32, name=f"t2{d}") for d in range(2)]
                nsc = [gpool.tile([128, 16], F32, name=f"n{d}") for d in range(2)]
                dsc = [gpool.tile([128, 16], F32, name=f"d{d}") for d in range(2)]
                zd = [gpool.tile([128, 16], F32, name=f"zd{d}") for d in range(2)]

                s = scope("gru_proj")
                nc.sync.dma_start(wg_t[:], wg_d[:])
                nc.sync.dma_start(whh_t[:], whh_d[:])
                nc.sync.dma_start(zmask_t[:], zmask_d[:])
                nc.vector.memset(ysb_f[:], 0.0)
                nc.vector.memset(ysb_b[:], 0.0)

                # ---- gru input projections: gx = x @ W_ih^T (+ z pad bias)
                with tc.For_i(0, BC) as bi:
                    with tc.For_i(0, 2 * OC6) as f:
                        nc.vector.tensor_copy(
                            wcopy2[:], wg_t[:, ds(f * 1024, 1024)]
                        )
                        for kc in range(KC):
                            nc.tensor.matmul(
                                psA2[:],
                                wcopy2[:, kc * 128:(kc + 1) * 128],
                                x_t[:, ds(kc * BC * T + bi * T, T)],
                                start=(kc == 0),
                                stop=(kc == KC - 1),
                            )
                        nc.vector.tensor_add(
                            gx_t[:, ds(f * BC * T + bi * T, T)],
                            psA2[:],
                            zmask_t[:, ds(f * BC * T + bi * T, T)],
                        )
                unscope(s)

                # ---- recurrence: fwd ascending t, bwd descending t ----
                s = scope("gru_rec")
                gxv = gx_t[:, :].rearrange(
                    "p (d c b t) -> p d c b t", d=2, c=OC6, b=BC
                )
                ysbv = [
                    y[:, :].rearrange("p (t q) -> p t q", q=16)
                    for y in (ysb_f, ysb_b)
                ]
                with tc.For_i(0, T) as j:
                    for dr, ysb in enumerate((ysb_f, ysb_b)):
                        tt = j if dr == 0 else (T - 1) - j       # time index
                        tr = j if dr == 0 else (T + 1) - j       # read block
                        tw = j + 1 if dr == 0 else T - j         # write block
                        # gate matmuls: gh = W_hh @ h_prev
                        for oc in range(OC6):
                            for kc in range(HC):
                                nc.tensor.matmul(
                                    ps_g[:, dr * 48 + oc * 8:dr * 48 + oc * 8 + 8],
                                    whh_t[:, (dr * OC6 * HC + oc * HC + kc) * 128:
                                          (dr * OC6 * HC + oc * HC + kc + 1) * 128],
                                    ysbv[dr][:, ds(tr, 1), kc * 8:(kc + 1) * 8]
                                    .rearrange("p a q -> p (a q)"),
                                    start=(kc == 0),
                                    stop=(kc == HC - 1),
                                )
                        gx_rz = (
                            gxv[:, dr, 0:4, :, :][:, :, :, ds(tt, 1)]
                            .rearrange("p c b o -> p (c b o)")
                        )
                        gx_n = (
                            gxv[:, dr, 4:6, :, :][:, :, :, ds(tt, 1)]
                            .rearrange("p c b o -> p (c b o)")
                        )
                        nc.vector.tensor_add(
                            grz[dr][:], ps_g[:, dr * 48:dr * 48 + 32], gx_rz
                        )
                        nc.scalar.activation(rz[dr][:], grz[dr][:], AF.Sigmoid)
                        nc.vector.tensor_mul(
                            t1[dr][:], rz[dr][:, 0:16],
                            ps_g[:, dr * 48 + 32:dr * 48 + 48],
                        )
                        nc.vector.tensor_add(t2[dr][:], t1[dr][:], gx_n)
                        nc.scalar.activation(nsc[dr][:], t2[dr][:], AF.Tanh)
                        hprev = (
                            ysbv[dr][:, ds(tr, 1), :].rearrange("p a q -> p (a q)")
                        )
                        nc.vector.tensor_sub(dsc[dr][:], hprev, nsc[dr][:])
                        nc.vector.tensor_mul(zd[dr][:], rz[dr][:, 16:32], dsc[dr][:])
                        nc.vector.tensor_add(
                            ysbv[dr][:, ds(tw, 1), :].rearrange("p a q -> p (a q)"),
                            nsc[dr][:], zd[dr][:],
                        )
                # zero the fwd tail (h carries past L; outputs there must be 0)
                nc.vector.tensor_mul(ysb_f[:], ysb_f[:], ymask_t[:])
                unscope(s)

        for _ in range(repeat):
            if phases in ("all", "attn"):
                attn_phase()
            if phases in ("all", "gru"):
                gru_phase()
            if phases == "attn":
                continue
            # ---- assembly: raw ysb dump; host does the final transpose ----
            s = scope("assembly")
            for dr, ysb in enumerate((ysb_f, ysb_b)):
                nc.sync.dma_start(yout[dr, :, :], ysb[:])
            unscope(s)

    nc.compile()
    return nc


_NC_CACHE = {}


def _get_nc(repeat: int = 1):
    if repeat not in _NC_CACHE:
        _NC_CACHE[repeat] = _build(repeat)
    return _NC_CACHE[repeat]


def _pack_w(w, ocn, kcn):
    # [ocn*128, kcn*128] -> [128, ocn*kcn*128] with blocks (oc, kc) of W^T
    a = np.asarray(w).reshape(ocn, 128, kcn, 128)      # (oc, j, kc, p)
    return np.ascontiguousarray(a.transpose(3, 0, 2, 1).reshape(128, -1))


def _host_inputs(inputs, core):
    bs = slice(core * BC, (core + 1) * BC)
    seg = np.asarray(inputs["seg_feats"][bs], dtype=np.float32)
    seglen = np.asarray(inputs["seglen"][bs]).astype(np.int64)

    for nm in ("bq", "bk", "bv", "bo", "b_ih_f", "b_hh_f", "b_ih_b", "b_hh_b"):
        assert not np.any(np.asarray(inputs[nm])), f"nonzero bias {nm} unsupported"

    m = {}
    xt = seg.transpose(2, 0, 1).reshape(KC, 128, BC, T)
    m["x_img"] = np.ascontiguousarray(
        xt.transpose(1, 0, 2, 3).reshape(128, -1)
    ).astype(ml_dtypes.bfloat16)

    for l in range(NL):
        blocks = [
            _pack_w(np.asarray(inputs[nm][l]), 8, 8).astype(ml_dtypes.bfloat16)
            for nm in ("Wq", "Wk", "Wv", "Wo")
        ]
        m[f"wl{l}"] = np.ascontiguousarray(np.concatenate(blocks, axis=1))

    m["wg"] = np.ascontiguousarray(np.concatenate(
        [
            _pack_w(np.asarray(inputs[nm]), OC6, KC).astype(ml_dtypes.bfloat16)
            for nm in ("W_ih_f", "W_ih_b")
        ],
        axis=1,
    ))
    m["whh"] = np.ascontiguousarray(np.concatenate(
        [
            _pack_w(np.asarray(inputs[nm]), OC6, HC).astype(np.float16)
            for nm in ("W_hh_f", "W_hh_b")
        ],
        axis=1,
    ))

    m["iden"] = np.eye(128, dtype=ml_dtypes.bfloat16)
    m["ones"] = np.ones((128, 128), dtype=ml_dtypes.bfloat16)

    p = np.arange(128)[:, None]
    q = np.arange(T)[None, :]
    band = np.concatenate(
        [
            np.where(np.abs((c * 128 + p) - q) <= ATTN_WIDTH, 0.0, -1e9)
            for c in range(2)
        ],
        axis=1,
    ).astype(np.float32)
    m["band"] = band.astype(ml_dtypes.bfloat16)

    t = np.arange(T)[None, :]
    pad = (t >= seglen[:, None]).astype(np.float32)          # [BC, T]
    zrow = np.zeros((2, OC6, BC, T), np.float32)
    zrow[:, 2] = 30.0 * pad
    zrow[:, 3] = 30.0 * pad
    m["zmask"] = np.broadcast_to(
        zrow.reshape(1, -1), (128, 2 * OC6 * BC * T)
    ).astype(ml_dtypes.bfloat16)

    ym = np.zeros((TB, HC, BC), np.float32)
    for b in range(BC):
        ym[1:T + 1, :, b] = (np.arange(T) < seglen[b]).astype(np.float32)[:, None]
    m["ymask"] = np.broadcast_to(
        ym.reshape(1, -1), (128, TB * 16)
    ).astype(np.float16)
    return m


def kernel(**inputs) -> np.ndarray:
    repeat = int(os.environ.get("KERNEL_REPEAT", "1"))
    nc = _get_nc(repeat)
    in_maps = [_host_inputs(inputs, c) for c in range(NCORES)]
    last_err = None
    for _ in range(3):
        try:
            res = run_bass_kernel_spmd(nc, in_maps, core_ids=list(range(NCORES)))
            break
        except Exception as e:  # transient NRT exec failures: retry
            last_err = e
    else:
        raise last_err
    outs = []
    for c in range(NCORES):
        raw = np.asarray(res.results[c]["yout"]).astype(np.float32)
        # raw[dr, p, (tb, hc, b)] -> y[b, t, dr*GH + hc*128 + p]
        r = raw.reshape(2, 128, TB, HC, BC)[:, :, 1:T + 1]
        outs.append(np.ascontiguousarray(r.transpose(4, 2, 0, 3, 1))
                    .reshape(BC, T, HID))
    return np.ascontiguousarray(np.concatenate(outs, axis=0))
